# revision 3
# baseline (speedup 1.0000x reference)
"""Trainium2 kernel for nn_IteratedLinearNet: y = x @ (W.T)^60.

Strategy (8 NeuronCores, single SPMD launch):
  - matrix power by squaring via the addition chain 2, 4, 8, 12, 24, 48, 60
    (7 matmuls of 2048^3 instead of 60 applications of x @ W.T)
  - each product is tensor-sharded: core j computes a 256-wide column slab
  - after each product (except the last) the core transposes its slab on
    TensorE and an 8-core AllGather assembles the full transposed matrix,
    which is the next product's stationary operand; AllGathers are split
    into column halves so compute pipelines with communication
  - final apply is tensor-parallel: core j computes y[:, Sj] for the full
    batch with x.T streamed from HBM
  - all matmuls run in float32r (FP22-truncated reads, full PE rate);
    inputs are pre-rounded to FP22-nearest on the host to keep the
    truncation exact and unbiased

Self-contained: builds/compiles on first call and caches the module.
"""

import numpy as np

_GRID = 2048
_BATCH = 4096
_NCORES = 8
_SW = _GRID // _NCORES  # 256
_KT = _GRID // 128  # 16
_HALF = _GRID // 2

# (power, lhsT_src, rhs_buf, out_buf); lhsT_src: "wt" or index of the step
# whose AllGather output (the transposed full matrix) is the stationary side.
_CHAIN = [
    (2, "wt", 0, 1),
    (4, 0, 1, 2),
    (8, 1, 2, 0),
    (12, 2, 2, 0),  # A12 = A8 @ A4 (rhs = A4 slab, still in buf 2)
    (24, 3, 0, 1),
    (48, 4, 1, 2),
    (60, 5, 0, 1),
]

_cache = {}


def _build():
    from contextlib import ExitStack

    import concourse.tile as tile
    from concourse import bacc, masks, mybir

    F32R = mybir.dt.float32r
    F32 = mybir.dt.float32
    G, KT, SW, HALF, BATCH = _GRID, _KT, _SW, _HALF, _BATCH

    nc = bacc.Bacc(None, target_bir_lowering=False, num_devices=_NCORES)
    wt = nc.declare_dram_parameter("wt", [G, G], F32R, isOutput=False)
    aslab = nc.declare_dram_parameter("aslab", [G, SW], F32R, isOutput=False)
    xt = nc.declare_dram_parameter("xt", [G, BATCH], F32R, isOutput=False)
    ytj = nc.declare_dram_parameter("ytj", [SW, BATCH], F32R, isOutput=True)

    rg = [list(range(_NCORES))]

    with ExitStack() as ctx:
        tc = ctx.enter_context(tile.TileContext(nc))
        big = ctx.enter_context(tc.tile_pool(name="big", bufs=1))
        slabs = ctx.enter_context(tc.tile_pool(name="slabs", bufs=1))
        shpool = ctx.enter_context(tc.tile_pool(name="shpool", bufs=3))
        ypool = ctx.enter_context(tc.tile_pool(name="ypool", bufs=2))
        mmps = ctx.enter_context(tc.tile_pool(name="mmps", bufs=4, space="PSUM"))
        tps = ctx.enter_context(tc.tile_pool(name="tps", bufs=2, space="PSUM"))
        dram = ctx.enter_context(tc.tile_pool(name="dram", bufs=2, space="DRAM"))

        lhsT_sb = big.tile([128, KT, G], F32R)
        sbuf = [
            slabs.tile([128, KT, SW], F32R, name=f"slab{i}", tag=f"slab{i}")
            for i in range(3)
        ]
        ident = slabs.tile([128, 128], F32, name="ident", tag="ident")
        masks.make_identity(nc, ident[:])

        for k in range(KT):
            nc.sync.dma_start(sbuf[0][:, k, :], aslab[128 * k : 128 * (k + 1), :])

        ag_outs = []
        n_steps = len(_CHAIN)
        for si, (power, src, rb, ob) in enumerate(_CHAIN):
            is_last = si == n_steps - 1
            rhs = sbuf[rb]
            out = sbuf[ob]
            ag_out_halves = []
            for h in range(2):
                for k in range(KT):
                    if src == "wt":
                        s_ap = wt[128 * k : 128 * (k + 1), HALF * h : HALF * (h + 1)]
                    else:
                        s_ap = ag_outs[src][h][128 * k : 128 * (k + 1), :]
                    nc.sync.dma_start(lhsT_sb[:, k, HALF * h : HALF * (h + 1)], s_ap)
                for m in range(8 * h, 8 * h + 8):
                    ps = mmps.tile([128, SW], F32, name="ps", tag="ps")
                    for k in range(KT):
                        nc.tensor.matmul(
                            ps[:],
                            lhsT_sb[:, k, 128 * m : 128 * (m + 1)],
                            rhs[:, k, :],
                            start=(k == 0),
                            stop=(k == KT - 1),
                        )
                    nc.vector.tensor_copy(out[:, m, :], ps[:])
                if is_last:
                    continue
                t_sb = shpool.tile([128, 2, HALF], F32R, name=f"t{si}_{h}", tag="sh8")
                for k in range(8 * h, 8 * h + 8):
                    for a in range(2):
                        psT = tps.tile([128, 128], F32R, name="psT", tag="psT")
                        nc.tensor.transpose(
                            psT[:], out[:, k, 128 * a : 128 * (a + 1)], ident[:]
                        )
                        nc.vector.tensor_copy(
                            t_sb[:, a, 128 * (k - 8 * h) : 128 * (k - 8 * h + 1)],
                            psT[:],
                        )
                ag_in = dram.tile([SW, HALF], F32R, name=f"agin{si}_{h}", tag="agin")
                for a in range(2):
                    nc.sync.dma_start(ag_in[128 * a : 128 * (a + 1), :], t_sb[:, a, :])
                ag_out = dram.tile(
                    [G, HALF],
                    F32R,
                    name=f"agout{si}_{h}",
                    tag="agout",
                    addr_space="Shared",
                )
                nc.gpsimd.collective_compute(
                    "AllGather",
                    mybir.AluOpType.bypass,
                    replica_groups=rg,
                    ins=[ag_in.opt()],
                    outs=[ag_out.opt()],
                )
                ag_out_halves.append(ag_out)
            ag_outs.append(ag_out_halves)

        final = sbuf[_CHAIN[-1][3]]
        for c in range(BATCH // SW):
            pss = [
                mmps.tile([128, SW], F32, name=f"psy{a}", tag="ps") for a in range(2)
            ]
            for kh in range(2):
                xchunk = shpool.tile([128, KT // 2, SW], F32R, name="xchunk", tag="sh8")
                for kk in range(KT // 2):
                    k = 8 * kh + kk
                    nc.sync.dma_start(
                        xchunk[:, kk, :],
                        xt[128 * k : 128 * (k + 1), SW * c : SW * (c + 1)],
                    )
                for a in range(2):
                    for kk in range(KT // 2):
                        k = 8 * kh + kk
                        nc.tensor.matmul(
                            pss[a][:],
                            final[:, k, 128 * a : 128 * (a + 1)],
                            xchunk[:, kk, :],
                            start=(k == 0),
                            stop=(k == KT - 1),
                        )
            for a in range(2):
                ystage = ypool.tile([128, SW], F32R, name="ystage", tag="ystage")
                nc.vector.tensor_copy(ystage[:], pss[a][:])
                nc.sync.dma_start(
                    ytj[128 * a : 128 * (a + 1), SW * c : SW * (c + 1)], ystage[:]
                )
    nc.compile()
    return nc


def _round22(a):
    bits = np.ascontiguousarray(a).view(np.uint32)
    return ((bits + 0x200) & np.uint32(0xFFFFFC00)).view(np.float32)


def kernel(x, W):
    from concourse.bass_utils import run_bass_kernel_spmd

    if "nc" not in _cache:
        _cache["nc"] = _build()
    nc = _cache["nc"]

    Wr = _round22(np.asarray(W, dtype=np.float32))
    xr = _round22(np.asarray(x, dtype=np.float32))
    wt_np = np.ascontiguousarray(Wr)
    xt_np = np.ascontiguousarray(xr.T)
    in_maps = [
        {
            "wt": wt_np,
            "aslab": np.ascontiguousarray(Wr[_SW * j : _SW * (j + 1), :].T),
            "xt": xt_np,
        }
        for j in range(_NCORES)
    ]
    res = run_bass_kernel_spmd(nc, in_maps, core_ids=list(range(_NCORES)))
    _cache["last_exec_time_ns"] = res.exec_time_ns
    _cache["last_results"] = res
    y = np.concatenate(
        [res.results[j]["ytj"].T for j in range(_NCORES)], axis=1
    ).astype(np.float32)
    return y
